# revision 15
# baseline (speedup 1.0000x reference)
"""Trainium2 Bass kernel for nn_EquivariantBiLinear.

Math (per batch row b):
    pieces:  Y[k, b] = sum_nu W_g[mu, nu] * x[b, bid_g[nu*r+rho]]   (k = off_g + mu*r + rho)
    out[b, o] = 0.1 * sum_i Y[W_invperm[o*256+i], b] * x[b, i]

Strategy: data-parallel over batch on 8 cores (weights replicated).
The group GEMMs produce Y in (128 k)-chunks that are consumed in place:
for each chunk, a one-hot matmul (RT, shipped) gathers the matching
0.1*xT rows, a DVE multiply forms Z = Y .* xTgather, and a second
one-hot matmul (O, built on GpSimd via iota-compare) scatter-reduces Z
into the 256 output columns accumulated in PSUM. The permutation never
touches DRAM and needs no row-granular DMA gathers.
"""

import sys

if "/opt/trn_rl_repo" not in sys.path:
    sys.path.insert(0, "/opt/trn_rl_repo")

from contextlib import ExitStack

import numpy as np

import concourse.bacc as bacc
import concourse.bass as bass
import concourse.mybir as mybir
import concourse.tile as tile
from concourse.bass import IndirectOffsetOnAxis
from concourse.bass_utils import run_bass_kernel_spmd
from concourse.masks import make_identity

GROUPS = [(512, 1, 16384), (256, 4, 4096), (128, 16, 1024), (64, 64, 256)]
OFF = [0, 16384, 32768, 49152]
X = 256
B = 2048
NCORES = 8
BS = B // NCORES  # 256 batch rows per core
NCHUNK = 512  # 65536 k-rows / 128

F32 = mybir.dt.float32
F32R = mybir.dt.float32r
BF16 = mybir.dt.bfloat16
FP16 = mybir.dt.float16
I32 = mybir.dt.int32


def _chunk_klists():
    """Global k indices (128 per chunk) in device production order."""
    ks = []
    p = np.arange(128)
    for mp in range(16):
        for mq in range(2):
            for sub in range(4):
                mt = mp * 8 + mq * 4 + sub
                ks.append(mt * 128 + p)
    for mt in range(32):
        for j in range(4):
            ks.append(16384 + (mt * 128 + p) * 4 + j)
    for mt in range(8):
        for np2 in range(4):
            for j in range(4):
                ks.append(32768 + (mt * 128 + p) * 16 + np2 * 4 + j)
    for mt in range(2):
        for s in range(2):
            for np3 in range(8):
                for j in range(4):
                    ks.append(49152 + (mt * 128 + p) * 64 + 2 * (np3 * 4 + j) + s)
    assert len(ks) == NCHUNK
    return ks


def _host_prep(W0, W1, W2, W3, bid0, bid1, bid2, bid3, W_invperm):
    """Pure layout transforms of weights/indices (no arithmetic on data)."""
    Ws = [np.asarray(W) for W in (W0, W1, W2, W3)]
    bids = [np.asarray(b).astype(np.int64) for b in (bid0, bid1, bid2, bid3)]
    wt = []
    for (n, r, m), W in zip(GROUPS, Ws):
        wt.append(np.ascontiguousarray(W.reshape(m, n).T.astype(np.float32)))
    wt3 = np.ascontiguousarray(np.concatenate([wt[3], wt[3]], axis=0))  # (128, 256)

    cols = []
    b0 = bids[0]
    for kc in range(4):
        cols.append(b0[kc * 128 : (kc + 1) * 128])
    b1 = bids[1].reshape(256, 4)
    for kc in range(2):
        for rho in range(4):
            cols.append(b1[kc * 128 : (kc + 1) * 128, rho])
    b2 = bids[2].reshape(128, 16)
    for rho in range(16):
        cols.append(b2[:, rho])
    b3 = bids[3].reshape(64, 64)
    p = np.arange(128)
    for q in range(32):
        cols.append(b3[p % 64, 2 * q + p // 64])
    xgidx = np.ascontiguousarray(np.stack(cols, axis=1).astype(np.int32))  # (128, 60)

    # inverse of W_invperm: perm[k] = o*256 + i position of Y row k
    ivp = np.asarray(W_invperm).astype(np.int64)
    perm = np.empty(X * X, np.int64)
    perm[ivp] = np.arange(X * X)

    klists = _chunk_klists()
    import ml_dtypes
    rt = np.zeros((X, X * X), np.float16)  # RT[i, col] one-hot
    oh = np.zeros((128, NCHUNK * X), np.float16)  # O[p, c*256+o] one-hot
    for c, kl in enumerate(klists):
        pk = perm[kl]
        rt[pk % X, c * 128 + np.arange(128)] = 1.0
        oh[np.arange(128), c * X + pk // X] = 1.0
    rt = np.ascontiguousarray(rt)
    oh = np.ascontiguousarray(oh)

    return wt[0], wt[1], wt[2], wt3, xgidx, rt, oh


def _build_nc():
    nc = bacc.Bacc("TRN2", target_bir_lowering=False, debug=False, num_devices=NCORES)

    xs_d = nc.dram_tensor("xs", [BS, X], F32, kind="ExternalInput")
    wt_d = [
        nc.dram_tensor("wt0", [512, 16384], F32R, kind="ExternalInput"),
        nc.dram_tensor("wt1", [256, 4096], F32R, kind="ExternalInput"),
        nc.dram_tensor("wt2", [128, 1024], F32R, kind="ExternalInput"),
        nc.dram_tensor("wt3", [128, 256], F32R, kind="ExternalInput"),
    ]
    xgidx_d = nc.dram_tensor("xgidx", [128, 60], I32, kind="ExternalInput")
    rt_d = nc.dram_tensor("rt", [X, X * X], FP16, kind="ExternalInput")
    oh_d = nc.dram_tensor("oh", [128, NCHUNK * X], FP16, kind="ExternalInput")
    out_d = nc.dram_tensor("out", [BS, X], F32, kind="ExternalOutput")

    with tile.TileContext(nc) as tc, ExitStack() as ctx:
        const = ctx.enter_context(tc.tile_pool(name="const", bufs=1))
        wpool = ctx.enter_context(tc.tile_pool(name="wpool", bufs=2))
        rtpool = ctx.enter_context(tc.tile_pool(name="rtpool", bufs=3))
        ypool = ctx.enter_context(tc.tile_pool(name="ypool", bufs=4))
        p2pool = ctx.enter_context(tc.tile_pool(name="p2pool", bufs=4))
        pgemm = ctx.enter_context(tc.tile_pool(name="pgemm", bufs=2, space="PSUM"))
        pxtg = ctx.enter_context(tc.tile_pool(name="pxtg", bufs=2, space="PSUM"))
        pout = ctx.enter_context(tc.tile_pool(name="pout", bufs=1, space="PSUM"))
        dram = ctx.enter_context(tc.tile_pool(name="dram", bufs=1, space="DRAM"))

        xT_dram = dram.tile([X, BS], F32R)

        ident = const.tile([128, 128], F32)
        make_identity(nc, ident[:])

        xgidx_t = const.tile([128, 60], I32)
        nc.sync.dma_start(xgidx_t[:], xgidx_d[:])


        # persistent output accumulators (one PSUM bank each)
        outT_ps = [
            pout.tile([128, BS], F32, tag=f"pout{oh}", name=f"pout{oh}")
            for oh in range(2)
        ]

        # ---- Phase 0: load x shard, transpose to xT (scaled) ----
        xs0 = const.tile([128, X], F32)  # batch rows 0..127
        xs1 = const.tile([128, X], F32)  # batch rows 128..255
        nc.sync.dma_start(xs0[:], xs_d[0:128, :])
        nc.sync.dma_start(xs1[:], xs_d[128:256, :])

        xtp0 = const.tile([128, BS], F32R)  # xT rows i=0..127 (plain, for xrep)
        xtp1 = const.tile([128, BS], F32R)  # xT rows i=128..255
        xts = const.tile([128, 512], FP16)  # [p, c*256+b] = 0.1*xT[c*128+p, b]
        for ih, xtp in ((0, xtp0), (1, xtp1)):
            for bh, xsrc in ((0, xs0), (1, xs1)):
                pst = pxtg.tile([128, 128], F32, tag="xtg", name="pst")
                nc.tensor.transpose(
                    pst[:], xsrc[:, ih * 128 : (ih + 1) * 128], ident[:]
                )
                nc.vector.tensor_copy(xtp[:, bh * 128 : (bh + 1) * 128], pst[:])
                nc.vector.tensor_scalar_mul(
                    xts[:, ih * 256 + bh * 128 : ih * 256 + (bh + 1) * 128],
                    pst[:],
                    0.1,
                )
        nc.sync.dma_start(xT_dram[0:128, :], xtp0[:])
        nc.sync.dma_start(xT_dram[128:256, :], xtp1[:])

        # ---- Phase 1a: gather x_rep tiles from xT via indirect DMA ----
        def igather(dst_slice, col):
            nc.gpsimd.indirect_dma_start(
                out=dst_slice,
                out_offset=None,
                in_=xT_dram[:],
                in_offset=IndirectOffsetOnAxis(ap=xgidx_t[:, col : col + 1], axis=0),
            )

        xrep0 = [const.tile([128, 256], F32R, tag=f"xrep0_{kc}", name=f"xrep0_{kc}") for kc in range(4)]
        for kc in range(4):
            igather(xrep0[kc][:], kc)
        xrep1 = [const.tile([128, 1024], F32R, tag=f"xrep1_{kc}", name=f"xrep1_{kc}") for kc in range(2)]
        for kc in range(2):
            for rho in range(4):
                igather(xrep1[kc][:, rho * 256 : (rho + 1) * 256], 4 + kc * 4 + rho)
        xrep2 = const.tile([128, 4096], F32R)
        for rho in range(16):
            igather(xrep2[:, rho * 256 : (rho + 1) * 256], 12 + rho)
        xrep3 = const.tile([128, 8192], F32R)
        for q in range(32):
            igather(xrep3[:, q * 256 : (q + 1) * 256], 28 + q)

        # ---- fused phase 2 machinery ----
        state = {"c": 0, "rt_lo": None, "rt_hi": None, "oht": None, "pend": []}

        def flush_pending():
            for obld, z, c in state["pend"]:
                for oh2 in range(2):
                    nc.tensor.matmul(
                        outT_ps[oh2][:],
                        obld[:, oh2 * 128 : (oh2 + 1) * 128],
                        z[:],
                        start=(c == 0),
                        stop=(c == NCHUNK - 1),
                        skip_group_check=True,
                    )
            state["pend"] = []

        def fuse_chunk(ysrc):
            """Consume one (128 k, 256 b) Y slice: xT-gather via RT matmul,
            Z = Y*xTg, O-matmul scatter-reduce into outT_ps."""
            c = state["c"]
            state["c"] = c + 1
            if len(state["pend"]) >= 8:
                flush_pending()
            if c % 8 == 0:
                rt_lo = rtpool.tile([128, 1024], FP16, tag="rt_lo", name="rt_lo")
                rt_hi = rtpool.tile([128, 1024], FP16, tag="rt_hi", name="rt_hi")
                nc.sync.dma_start(rt_lo[:], rt_d[0:128, c * 128 : (c + 8) * 128])
                nc.sync.dma_start(rt_hi[:], rt_d[128:256, c * 128 : (c + 8) * 128])
                state["rt_lo"], state["rt_hi"] = rt_lo, rt_hi
                oht = rtpool.tile([128, 2048], FP16, tag="oht", name="oht")
                nc.sync.dma_start(oht[:], oh_d[:, c * 256 : (c + 8) * 256])
                state["oht"] = oht
            rts = (state["rt_lo"], state["rt_hi"])
            lo = (c % 8) * 128
            xtg = pxtg.tile([128, 256], F32, tag="xtg", name="xtg")
            for ih in range(2):
                nc.tensor.matmul(
                    xtg[:],
                    rts[ih][:, lo : lo + 128],
                    xts[:, ih * 256 : (ih + 1) * 256],
                    start=(ih == 0),
                    stop=(ih == 1),
                )
            obld = state["oht"][:, (c % 8) * 256 : (c % 8 + 1) * 256]
            z = p2pool.tile([128, 256], FP16, tag="z", name="z", bufs=14)
            nc.vector.tensor_mul(z[:], ysrc, xtg[:])
            state["pend"].append((obld, z, c))

        copy_eng = [lambda o, i: nc.vector.tensor_copy(o, i),
                    lambda o, i: nc.scalar.copy(o, i)]
        state_cp = {"n": 0}

        def psum_to_sbuf(ps_ap):
            yt = ypool.tile([128, 1024], F32, tag="ytile", name="yt")
            copy_eng[state_cp["n"] % 2](yt[:], ps_ap)
            state_cp["n"] += 1
            return yt

        # ---- Phase 1b+2: group GEMMs with fused consumption ----
        # g0: k = mu
        for mp in range(16):
            w0t = [wpool.tile([128, 1024], F32R, tag=f"w0_{kc}", name=f"w0_{kc}") for kc in range(4)]
            for kc in range(4):
                nc.sync.dma_start(
                    w0t[kc][:],
                    wt_d[0][kc * 128 : (kc + 1) * 128, mp * 1024 : (mp + 1) * 1024],
                )
            for mq in range(2):
                ps = pgemm.tile([128, 1024], F32, tag="pg", name="ps")
                for sub in range(4):
                    mt = mq * 4 + sub
                    for kc in range(4):
                        nc.tensor.matmul(
                            ps[:, sub * 256 : (sub + 1) * 256],
                            w0t[kc][:, mt * 128 : (mt + 1) * 128],
                            xrep0[kc][:],
                            start=(kc == 0),
                            stop=(kc == 3),
                        )
                yt = psum_to_sbuf(ps[:])
                for sub in range(4):
                    fuse_chunk(yt[:, sub * 256 : (sub + 1) * 256])

        # g1: k = 16384 + mu*4 + rho
        w1t = [const.tile([128, 4096], F32R, tag=f"w1_{kc}", name=f"w1_{kc}") for kc in range(2)]
        for kc in range(2):
            nc.sync.dma_start(w1t[kc][:], wt_d[1][kc * 128 : (kc + 1) * 128, :])
        for mt in range(32):
            ps = pgemm.tile([128, 1024], F32, tag="pg", name="ps")
            for ns in range(2):
                for kc in range(2):
                    nc.tensor.matmul(
                        ps[:, ns * 512 : (ns + 1) * 512],
                        w1t[kc][:, mt * 128 : (mt + 1) * 128],
                        xrep1[kc][:, ns * 512 : (ns + 1) * 512],
                        start=(kc == 0),
                        stop=(kc == 1),
                    )
            yt = psum_to_sbuf(ps[:])
            for j in range(4):
                fuse_chunk(yt[:, j * 256 : (j + 1) * 256])

        # g2: k = 32768 + mu*16 + rho
        w2t = const.tile([128, 1024], F32R)
        nc.sync.dma_start(w2t[:], wt_d[2][:])
        for mt in range(8):
            for np2 in range(4):
                ps = pgemm.tile([128, 1024], F32, tag="pg", name="ps")
                for sub in range(2):
                    ns = np2 * 2 + sub
                    nc.tensor.matmul(
                        ps[:, sub * 512 : (sub + 1) * 512],
                        w2t[:, mt * 128 : (mt + 1) * 128],
                        xrep2[:, ns * 512 : (ns + 1) * 512],
                        start=True,
                        stop=True,
                    )
                yt = psum_to_sbuf(ps[:])
                for j in range(4):
                    fuse_chunk(yt[:, j * 256 : (j + 1) * 256])

        # g3: k = 49152 + mu*64 + 2q + s
        w3t = const.tile([128, 256], F32R)
        nc.sync.dma_start(w3t[:], wt_d[3][:])
        for mt in range(2):
            for s in range(2):
                for np3 in range(8):
                    ps = pgemm.tile([128, 1024], F32, tag="pg", name="ps")
                    for sub in range(2):
                        ns = np3 * 2 + sub
                        nc.tensor.matmul(
                            ps[:, sub * 512 : (sub + 1) * 512],
                            w3t[s * 64 : (s + 1) * 64, mt * 128 : (mt + 1) * 128],
                            xrep3[s * 64 : (s + 1) * 64, ns * 512 : (ns + 1) * 512],
                            start=True,
                            stop=True,
                        )
                    yt = psum_to_sbuf(ps[:])
                    for j in range(4):
                        fuse_chunk(yt[:, j * 256 : (j + 1) * 256])

        assert state["c"] == NCHUNK
        flush_pending()

        # ---- epilogue: outT (o, b) -> out (b, o) ----
        outstage = [const.tile([128, 256], F32, tag=f"outstage{bh}", name=f"outstage{bh}") for bh in range(2)]
        for oh in range(2):
            outT_sb = p2pool.tile([128, 256], F32, tag="outT_sb", name="outT_sb", bufs=2)
            nc.vector.tensor_copy(outT_sb[:], outT_ps[oh][:])
            for bh in range(2):
                pst2 = pxtg.tile([128, 128], F32, tag="xtg", name="pst2")
                nc.tensor.transpose(
                    pst2[:], outT_sb[:, bh * 128 : (bh + 1) * 128], ident[:]
                )
                nc.any.tensor_copy(
                    outstage[bh][:, oh * 128 : (oh + 1) * 128], pst2[:]
                )
        for bh in range(2):
            nc.sync.dma_start(out_d[bh * 128 : (bh + 1) * 128, :], outstage[bh][:])

    nc.compile()
    return nc


_NC_CACHE = None


def _make_in_maps(x, wt0, wt1, wt2, wt3, xgidx, rt, oh):
    x = np.ascontiguousarray(np.asarray(x, dtype=np.float32))
    in_maps = []
    for c in range(NCORES):
        in_maps.append(
            {
                "xs": x[c * BS : (c + 1) * BS, :],
                "wt0": wt0,
                "wt1": wt1,
                "wt2": wt2,
                "wt3": wt3,
                "xgidx": xgidx,
                "rt": rt,
                "oh": oh,
            }
        )
    return in_maps


def kernel(x, W0, W1, W2, W3, bid0, bid1, bid2, bid3, W_invperm, **_unused):
    global _NC_CACHE
    prep = _host_prep(W0, W1, W2, W3, bid0, bid1, bid2, bid3, W_invperm)
    if _NC_CACHE is None:
        _NC_CACHE = _build_nc()
    nc = _NC_CACHE

    in_maps = _make_in_maps(x, *prep)
    res = run_bass_kernel_spmd(nc, in_maps, core_ids=list(range(NCORES)))
    out = np.concatenate([res.results[c]["out"] for c in range(NCORES)], axis=0)
    return out.astype(np.float32)


# revision 16
# speedup vs baseline: 1.1541x; 1.1541x over previous
"""Trainium2 Bass kernel for nn_EquivariantBiLinear.

Math (per batch row b):
    pieces:  Y[k, b] = sum_nu W_g[mu, nu] * x[b, bid_g[nu*r+rho]]   (k = off_g + mu*r + rho)
    out[b, o] = 0.1 * sum_i Y[W_invperm[o*256+i], b] * x[b, i]

Strategy: data-parallel over batch on 8 cores (weights replicated).
The group GEMMs produce Y in (128 k)-chunks that are consumed in place:
for each chunk, a one-hot matmul (RT, shipped) gathers the matching
0.1*xT rows, a DVE multiply forms Z = Y .* xTgather, and a second
one-hot matmul (O, built on GpSimd via iota-compare) scatter-reduces Z
into the 256 output columns accumulated in PSUM. The permutation never
touches DRAM and needs no row-granular DMA gathers.
"""

import sys

if "/opt/trn_rl_repo" not in sys.path:
    sys.path.insert(0, "/opt/trn_rl_repo")

from contextlib import ExitStack

import numpy as np

import concourse.bacc as bacc
import concourse.bass as bass
import concourse.mybir as mybir
import concourse.tile as tile
from concourse.bass import IndirectOffsetOnAxis
from concourse.bass_utils import run_bass_kernel_spmd
from concourse.masks import make_identity

GROUPS = [(512, 1, 16384), (256, 4, 4096), (128, 16, 1024), (64, 64, 256)]
OFF = [0, 16384, 32768, 49152]
X = 256
B = 2048
NCORES = 8
BS = B // NCORES  # 256 batch rows per core
NCHUNK = 512  # 65536 k-rows / 128

F32 = mybir.dt.float32
F32R = mybir.dt.float32r
BF16 = mybir.dt.bfloat16
FP16 = mybir.dt.float16
I32 = mybir.dt.int32


def _chunk_klists():
    """Global k indices (128 per chunk) in device production order."""
    ks = []
    p = np.arange(128)
    for mp in range(16):
        for mq in range(2):
            for sub in range(4):
                mt = mp * 8 + mq * 4 + sub
                ks.append(mt * 128 + p)
    for mt in range(32):
        for j in range(4):
            ks.append(16384 + (mt * 128 + p) * 4 + j)
    for mt in range(8):
        for np2 in range(4):
            for j in range(4):
                ks.append(32768 + (mt * 128 + p) * 16 + np2 * 4 + j)
    for mt in range(2):
        for s in range(2):
            for np3 in range(8):
                for j in range(4):
                    ks.append(49152 + (mt * 128 + p) * 64 + 2 * (np3 * 4 + j) + s)
    assert len(ks) == NCHUNK
    return ks


def _host_prep(W0, W1, W2, W3, bid0, bid1, bid2, bid3, W_invperm):
    """Pure layout transforms of weights/indices (no arithmetic on data)."""
    Ws = [np.asarray(W) for W in (W0, W1, W2, W3)]
    bids = [np.asarray(b).astype(np.int64) for b in (bid0, bid1, bid2, bid3)]
    wt = []
    for (n, r, m), W in zip(GROUPS, Ws):
        wt.append(np.ascontiguousarray(W.reshape(m, n).T.astype(np.float16)))
    wt3 = np.ascontiguousarray(np.concatenate([wt[3], wt[3]], axis=0))  # (128, 256)

    cols = []
    b0 = bids[0]
    for kc in range(4):
        cols.append(b0[kc * 128 : (kc + 1) * 128])
    b1 = bids[1].reshape(256, 4)
    for kc in range(2):
        for rho in range(4):
            cols.append(b1[kc * 128 : (kc + 1) * 128, rho])
    b2 = bids[2].reshape(128, 16)
    for rho in range(16):
        cols.append(b2[:, rho])
    b3 = bids[3].reshape(64, 64)
    p = np.arange(128)
    for q in range(32):
        cols.append(b3[p % 64, 2 * q + p // 64])
    xgidx = np.ascontiguousarray(np.stack(cols, axis=1).astype(np.int32))  # (128, 60)

    # inverse of W_invperm: perm[k] = o*256 + i position of Y row k
    ivp = np.asarray(W_invperm).astype(np.int64)
    perm = np.empty(X * X, np.int64)
    perm[ivp] = np.arange(X * X)

    klists = _chunk_klists()
    import ml_dtypes
    rt = np.zeros((X, X * X), np.float16)  # RT[i, col] one-hot
    oh = np.zeros((128, NCHUNK * X), np.float16)  # O[p, c*256+o] one-hot
    for c, kl in enumerate(klists):
        pk = perm[kl]
        rt[pk % X, c * 128 + np.arange(128)] = 1.0
        oh[np.arange(128), c * X + pk // X] = 1.0
    rt = np.ascontiguousarray(rt)
    oh = np.ascontiguousarray(oh)

    return wt[0], wt[1], wt[2], wt3, xgidx, rt, oh


def _build_nc():
    nc = bacc.Bacc("TRN2", target_bir_lowering=False, debug=False, num_devices=NCORES)

    xs_d = nc.dram_tensor("xs", [BS, X], F32, kind="ExternalInput")
    wt_d = [
        nc.dram_tensor("wt0", [512, 16384], FP16, kind="ExternalInput"),
        nc.dram_tensor("wt1", [256, 4096], FP16, kind="ExternalInput"),
        nc.dram_tensor("wt2", [128, 1024], FP16, kind="ExternalInput"),
        nc.dram_tensor("wt3", [128, 256], FP16, kind="ExternalInput"),
    ]
    xgidx_d = nc.dram_tensor("xgidx", [128, 60], I32, kind="ExternalInput")
    rt_d = nc.dram_tensor("rt", [X, X * X], FP16, kind="ExternalInput")
    oh_d = nc.dram_tensor("oh", [128, NCHUNK * X], FP16, kind="ExternalInput")
    out_d = nc.dram_tensor("out", [BS, X], F32, kind="ExternalOutput")

    with tile.TileContext(nc) as tc, ExitStack() as ctx:
        const = ctx.enter_context(tc.tile_pool(name="const", bufs=1))
        wpool = ctx.enter_context(tc.tile_pool(name="wpool", bufs=2))
        rtpool = ctx.enter_context(tc.tile_pool(name="rtpool", bufs=3))
        ypool = ctx.enter_context(tc.tile_pool(name="ypool", bufs=4))
        p2pool = ctx.enter_context(tc.tile_pool(name="p2pool", bufs=4))
        pgemm = ctx.enter_context(tc.tile_pool(name="pgemm", bufs=2, space="PSUM"))
        pxtg = ctx.enter_context(tc.tile_pool(name="pxtg", bufs=2, space="PSUM"))
        pout = ctx.enter_context(tc.tile_pool(name="pout", bufs=1, space="PSUM"))
        dram = ctx.enter_context(tc.tile_pool(name="dram", bufs=1, space="DRAM"))

        xT_dram = dram.tile([X, BS], FP16)

        ident = const.tile([128, 128], F32)
        make_identity(nc, ident[:])

        xgidx_t = const.tile([128, 60], I32)
        nc.sync.dma_start(xgidx_t[:], xgidx_d[:])


        # persistent output accumulators (one PSUM bank each)
        outT_ps = [
            pout.tile([128, BS], F32, tag=f"pout{oh}", name=f"pout{oh}")
            for oh in range(2)
        ]

        # ---- Phase 0: load x shard, transpose to xT (scaled) ----
        xs0 = const.tile([128, X], F32)  # batch rows 0..127
        xs1 = const.tile([128, X], F32)  # batch rows 128..255
        nc.sync.dma_start(xs0[:], xs_d[0:128, :])
        nc.sync.dma_start(xs1[:], xs_d[128:256, :])

        xtp0 = const.tile([128, BS], FP16)  # xT rows i=0..127 (plain, for xrep)
        xtp1 = const.tile([128, BS], FP16)  # xT rows i=128..255
        xts = const.tile([128, 512], FP16)  # [p, c*256+b] = 0.1*xT[c*128+p, b]
        for ih, xtp in ((0, xtp0), (1, xtp1)):
            for bh, xsrc in ((0, xs0), (1, xs1)):
                pst = pxtg.tile([128, 128], F32, tag="xtg", name="pst")
                nc.tensor.transpose(
                    pst[:], xsrc[:, ih * 128 : (ih + 1) * 128], ident[:]
                )
                nc.vector.tensor_copy(xtp[:, bh * 128 : (bh + 1) * 128], pst[:])
                nc.vector.tensor_scalar_mul(
                    xts[:, ih * 256 + bh * 128 : ih * 256 + (bh + 1) * 128],
                    pst[:],
                    0.1,
                )
        nc.sync.dma_start(xT_dram[0:128, :], xtp0[:])
        nc.sync.dma_start(xT_dram[128:256, :], xtp1[:])

        # ---- Phase 1a: gather x_rep tiles from xT via indirect DMA ----
        def igather(dst_slice, col):
            nc.gpsimd.indirect_dma_start(
                out=dst_slice,
                out_offset=None,
                in_=xT_dram[:],
                in_offset=IndirectOffsetOnAxis(ap=xgidx_t[:, col : col + 1], axis=0),
            )

        xrep0 = [const.tile([128, 256], FP16, tag=f"xrep0_{kc}", name=f"xrep0_{kc}") for kc in range(4)]
        for kc in range(4):
            igather(xrep0[kc][:], kc)
        xrep1 = [const.tile([128, 1024], FP16, tag=f"xrep1_{kc}", name=f"xrep1_{kc}") for kc in range(2)]
        for kc in range(2):
            for rho in range(4):
                igather(xrep1[kc][:, rho * 256 : (rho + 1) * 256], 4 + kc * 4 + rho)
        xrep2 = const.tile([128, 4096], FP16)
        for rho in range(16):
            igather(xrep2[:, rho * 256 : (rho + 1) * 256], 12 + rho)
        xrep3 = const.tile([128, 8192], FP16)
        for q in range(32):
            igather(xrep3[:, q * 256 : (q + 1) * 256], 28 + q)

        # ---- fused phase 2 machinery ----
        state = {"c": 0, "rt_lo": None, "rt_hi": None, "oht": None, "pend": []}

        def flush_pending():
            for obld, z, c in state["pend"]:
                for oh2 in range(2):
                    nc.tensor.matmul(
                        outT_ps[oh2][:],
                        obld[:, oh2 * 128 : (oh2 + 1) * 128],
                        z[:],
                        start=(c == 0),
                        stop=(c == NCHUNK - 1),
                        skip_group_check=True,
                    )
            state["pend"] = []

        def fuse_chunk(ysrc):
            """Consume one (128 k, 256 b) Y slice: xT-gather via RT matmul,
            Z = Y*xTg, O-matmul scatter-reduce into outT_ps."""
            c = state["c"]
            state["c"] = c + 1
            if len(state["pend"]) >= 8:
                flush_pending()
            if c % 8 == 0:
                rt_lo = rtpool.tile([128, 1024], FP16, tag="rt_lo", name="rt_lo")
                rt_hi = rtpool.tile([128, 1024], FP16, tag="rt_hi", name="rt_hi")
                nc.sync.dma_start(rt_lo[:], rt_d[0:128, c * 128 : (c + 8) * 128])
                nc.sync.dma_start(rt_hi[:], rt_d[128:256, c * 128 : (c + 8) * 128])
                state["rt_lo"], state["rt_hi"] = rt_lo, rt_hi
                oht = rtpool.tile([128, 2048], FP16, tag="oht", name="oht")
                nc.sync.dma_start(oht[:], oh_d[:, c * 256 : (c + 8) * 256])
                state["oht"] = oht
            rts = (state["rt_lo"], state["rt_hi"])
            lo = (c % 8) * 128
            xtg = pxtg.tile([128, 256], F32, tag="xtg", name="xtg")
            for ih in range(2):
                nc.tensor.matmul(
                    xtg[:],
                    rts[ih][:, lo : lo + 128],
                    xts[:, ih * 256 : (ih + 1) * 256],
                    start=(ih == 0),
                    stop=(ih == 1),
                )
            obld = state["oht"][:, (c % 8) * 256 : (c % 8 + 1) * 256]
            z = p2pool.tile([128, 256], FP16, tag="z", name="z", bufs=14)
            nc.vector.tensor_mul(z[:], ysrc, xtg[:])
            state["pend"].append((obld, z, c))

        copy_eng = [lambda o, i: nc.vector.tensor_copy(o, i),
                    lambda o, i: nc.scalar.copy(o, i)]
        state_cp = {"n": 0}

        def psum_to_sbuf(ps_ap):
            yt = ypool.tile([128, 1024], F32, tag="ytile", name="yt")
            copy_eng[state_cp["n"] % 2](yt[:], ps_ap)
            state_cp["n"] += 1
            return yt

        # ---- Phase 1b+2: group GEMMs with fused consumption ----
        # g0: k = mu
        for mp in range(16):
            w0t = [wpool.tile([128, 1024], FP16, tag=f"w0_{kc}", name=f"w0_{kc}") for kc in range(4)]
            for kc in range(4):
                nc.sync.dma_start(
                    w0t[kc][:],
                    wt_d[0][kc * 128 : (kc + 1) * 128, mp * 1024 : (mp + 1) * 1024],
                )
            for mq in range(2):
                ps = pgemm.tile([128, 1024], F32, tag="pg", name="ps")
                for sub in range(4):
                    mt = mq * 4 + sub
                    for kc in range(4):
                        nc.tensor.matmul(
                            ps[:, sub * 256 : (sub + 1) * 256],
                            w0t[kc][:, mt * 128 : (mt + 1) * 128],
                            xrep0[kc][:],
                            start=(kc == 0),
                            stop=(kc == 3),
                        )
                yt = psum_to_sbuf(ps[:])
                for sub in range(4):
                    fuse_chunk(yt[:, sub * 256 : (sub + 1) * 256])

        # g1: k = 16384 + mu*4 + rho
        w1t = [const.tile([128, 4096], FP16, tag=f"w1_{kc}", name=f"w1_{kc}") for kc in range(2)]
        for kc in range(2):
            nc.sync.dma_start(w1t[kc][:], wt_d[1][kc * 128 : (kc + 1) * 128, :])
        for mt in range(32):
            ps = pgemm.tile([128, 1024], F32, tag="pg", name="ps")
            for ns in range(2):
                for kc in range(2):
                    nc.tensor.matmul(
                        ps[:, ns * 512 : (ns + 1) * 512],
                        w1t[kc][:, mt * 128 : (mt + 1) * 128],
                        xrep1[kc][:, ns * 512 : (ns + 1) * 512],
                        start=(kc == 0),
                        stop=(kc == 1),
                    )
            yt = psum_to_sbuf(ps[:])
            for j in range(4):
                fuse_chunk(yt[:, j * 256 : (j + 1) * 256])

        # g2: k = 32768 + mu*16 + rho
        w2t = const.tile([128, 1024], FP16)
        nc.sync.dma_start(w2t[:], wt_d[2][:])
        for mt in range(8):
            for np2 in range(4):
                ps = pgemm.tile([128, 1024], F32, tag="pg", name="ps")
                for sub in range(2):
                    ns = np2 * 2 + sub
                    nc.tensor.matmul(
                        ps[:, sub * 512 : (sub + 1) * 512],
                        w2t[:, mt * 128 : (mt + 1) * 128],
                        xrep2[:, ns * 512 : (ns + 1) * 512],
                        start=True,
                        stop=True,
                    )
                yt = psum_to_sbuf(ps[:])
                for j in range(4):
                    fuse_chunk(yt[:, j * 256 : (j + 1) * 256])

        # g3: k = 49152 + mu*64 + 2q + s
        w3t = const.tile([128, 256], FP16)
        nc.sync.dma_start(w3t[:], wt_d[3][:])
        for mt in range(2):
            for s in range(2):
                for np3 in range(8):
                    ps = pgemm.tile([128, 1024], F32, tag="pg", name="ps")
                    for sub in range(2):
                        ns = np3 * 2 + sub
                        nc.tensor.matmul(
                            ps[:, sub * 512 : (sub + 1) * 512],
                            w3t[s * 64 : (s + 1) * 64, mt * 128 : (mt + 1) * 128],
                            xrep3[s * 64 : (s + 1) * 64, ns * 512 : (ns + 1) * 512],
                            start=True,
                            stop=True,
                        )
                    yt = psum_to_sbuf(ps[:])
                    for j in range(4):
                        fuse_chunk(yt[:, j * 256 : (j + 1) * 256])

        assert state["c"] == NCHUNK
        flush_pending()

        # ---- epilogue: outT (o, b) -> out (b, o) ----
        outstage = [const.tile([128, 256], F32, tag=f"outstage{bh}", name=f"outstage{bh}") for bh in range(2)]
        for oh in range(2):
            outT_sb = p2pool.tile([128, 256], F32, tag="outT_sb", name="outT_sb", bufs=2)
            nc.vector.tensor_copy(outT_sb[:], outT_ps[oh][:])
            for bh in range(2):
                pst2 = pxtg.tile([128, 128], F32, tag="xtg", name="pst2")
                nc.tensor.transpose(
                    pst2[:], outT_sb[:, bh * 128 : (bh + 1) * 128], ident[:]
                )
                nc.any.tensor_copy(
                    outstage[bh][:, oh * 128 : (oh + 1) * 128], pst2[:]
                )
        for bh in range(2):
            nc.sync.dma_start(out_d[bh * 128 : (bh + 1) * 128, :], outstage[bh][:])

    nc.compile()
    return nc


_NC_CACHE = None


def _make_in_maps(x, wt0, wt1, wt2, wt3, xgidx, rt, oh):
    x = np.ascontiguousarray(np.asarray(x, dtype=np.float32))
    in_maps = []
    for c in range(NCORES):
        in_maps.append(
            {
                "xs": x[c * BS : (c + 1) * BS, :],
                "wt0": wt0,
                "wt1": wt1,
                "wt2": wt2,
                "wt3": wt3,
                "xgidx": xgidx,
                "rt": rt,
                "oh": oh,
            }
        )
    return in_maps


def kernel(x, W0, W1, W2, W3, bid0, bid1, bid2, bid3, W_invperm, **_unused):
    global _NC_CACHE
    prep = _host_prep(W0, W1, W2, W3, bid0, bid1, bid2, bid3, W_invperm)
    if _NC_CACHE is None:
        _NC_CACHE = _build_nc()
    nc = _NC_CACHE

    in_maps = _make_in_maps(x, *prep)
    res = run_bass_kernel_spmd(nc, in_maps, core_ids=list(range(NCORES)))
    out = np.concatenate([res.results[c]["out"] for c in range(NCORES)], axis=0)
    return out.astype(np.float32)


# revision 17
# speedup vs baseline: 1.1775x; 1.0203x over previous
"""Trainium2 Bass kernel for nn_EquivariantBiLinear.

Math (per batch row b):
    pieces:  Y[k, b] = sum_nu W_g[mu, nu] * x[b, bid_g[nu*r+rho]]   (k = off_g + mu*r + rho)
    out[b, o] = 0.1 * sum_i Y[W_invperm[o*256+i], b] * x[b, i]

Strategy: data-parallel over batch on 8 cores (weights replicated).
The group GEMMs produce Y in (128 k)-chunks that are consumed in place:
for each chunk, a one-hot matmul (RT, shipped) gathers the matching
0.1*xT rows, a DVE multiply forms Z = Y .* xTgather, and a second
one-hot matmul (O, built on GpSimd via iota-compare) scatter-reduces Z
into the 256 output columns accumulated in PSUM. The permutation never
touches DRAM and needs no row-granular DMA gathers.
"""

import sys

if "/opt/trn_rl_repo" not in sys.path:
    sys.path.insert(0, "/opt/trn_rl_repo")

from contextlib import ExitStack

import numpy as np

import concourse.bacc as bacc
import concourse.bass as bass
import concourse.mybir as mybir
import concourse.tile as tile
from concourse.bass import IndirectOffsetOnAxis
from concourse.bass_utils import run_bass_kernel_spmd
from concourse.masks import make_identity

GROUPS = [(512, 1, 16384), (256, 4, 4096), (128, 16, 1024), (64, 64, 256)]
OFF = [0, 16384, 32768, 49152]
X = 256
B = 2048
NCORES = 8
BS = B // NCORES  # 256 batch rows per core
NCHUNK = 512  # 65536 k-rows / 128

F32 = mybir.dt.float32
F32R = mybir.dt.float32r
BF16 = mybir.dt.bfloat16
FP16 = mybir.dt.float16
I32 = mybir.dt.int32


def _chunk_klists():
    """Global k indices (128 per chunk) in device production order."""
    ks = []
    p = np.arange(128)
    for mp in range(16):
        for mq in range(2):
            for sub in range(4):
                mt = mp * 8 + mq * 4 + sub
                ks.append(mt * 128 + p)
    for mt in range(32):
        for j in range(4):
            ks.append(16384 + (mt * 128 + p) * 4 + j)
    for mt in range(8):
        for np2 in range(4):
            for j in range(4):
                ks.append(32768 + (mt * 128 + p) * 16 + np2 * 4 + j)
    for mt in range(2):
        for s in range(2):
            for np3 in range(8):
                for j in range(4):
                    ks.append(49152 + (mt * 128 + p) * 64 + 2 * (np3 * 4 + j) + s)
    assert len(ks) == NCHUNK
    return ks


def _host_prep(W0, W1, W2, W3, bid0, bid1, bid2, bid3, W_invperm):
    """Pure layout transforms of weights/indices (no arithmetic on data)."""
    Ws = [np.asarray(W) for W in (W0, W1, W2, W3)]
    bids = [np.asarray(b).astype(np.int64) for b in (bid0, bid1, bid2, bid3)]
    wt = []
    for (n, r, m), W in zip(GROUPS, Ws):
        wt.append(np.ascontiguousarray(W.reshape(m, n).T.astype(np.float16)))
    wt3 = np.ascontiguousarray(np.concatenate([wt[3], wt[3]], axis=0))  # (128, 256)

    cols = []
    b0 = bids[0]
    for kc in range(4):
        cols.append(b0[kc * 128 : (kc + 1) * 128])
    b1 = bids[1].reshape(256, 4)
    for kc in range(2):
        for rho in range(4):
            cols.append(b1[kc * 128 : (kc + 1) * 128, rho])
    b2 = bids[2].reshape(128, 16)
    for rho in range(16):
        cols.append(b2[:, rho])
    b3 = bids[3].reshape(64, 64)
    p = np.arange(128)
    for q in range(32):
        cols.append(b3[p % 64, 2 * q + p // 64])
    xgidx = np.ascontiguousarray(np.stack(cols, axis=1).astype(np.int32))  # (128, 60)

    # inverse of W_invperm: perm[k] = o*256 + i position of Y row k
    ivp = np.asarray(W_invperm).astype(np.int64)
    perm = np.empty(X * X, np.int64)
    perm[ivp] = np.arange(X * X)

    klists = _chunk_klists()
    import ml_dtypes
    rt = np.zeros((X, X * X), np.float16)  # RT[i, col] one-hot
    oh = np.zeros((128, NCHUNK * X), np.float16)  # O[p, c*256+o] one-hot
    for c, kl in enumerate(klists):
        pk = perm[kl]
        rt[pk % X, c * 128 + np.arange(128)] = 1.0
        oh[np.arange(128), c * X + pk // X] = 1.0
    rt = np.ascontiguousarray(rt)
    oh = np.ascontiguousarray(oh)

    return wt[0], wt[1], wt[2], wt3, xgidx, rt, oh


def _build_nc():
    nc = bacc.Bacc("TRN2", target_bir_lowering=False, debug=False, num_devices=NCORES)

    xs_d = nc.dram_tensor("xs", [BS, X], F32, kind="ExternalInput")
    xt_d = nc.dram_tensor("xt", [X, BS], FP16, kind="ExternalInput")
    wt_d = [
        nc.dram_tensor("wt0", [512, 16384], FP16, kind="ExternalInput"),
        nc.dram_tensor("wt1", [256, 4096], FP16, kind="ExternalInput"),
        nc.dram_tensor("wt2", [128, 1024], FP16, kind="ExternalInput"),
        nc.dram_tensor("wt3", [128, 256], FP16, kind="ExternalInput"),
    ]
    xgidx_d = nc.dram_tensor("xgidx", [128, 60], I32, kind="ExternalInput")
    rt_d = nc.dram_tensor("rt", [X, X * X], FP16, kind="ExternalInput")
    oh_d = nc.dram_tensor("oh", [128, NCHUNK * X], FP16, kind="ExternalInput")
    out_d = nc.dram_tensor("out", [BS, X], F32, kind="ExternalOutput")

    with tile.TileContext(nc) as tc, ExitStack() as ctx:
        const = ctx.enter_context(tc.tile_pool(name="const", bufs=1))
        wpool = ctx.enter_context(tc.tile_pool(name="wpool", bufs=3))
        rtpool = ctx.enter_context(tc.tile_pool(name="rtpool", bufs=4))
        ypool = ctx.enter_context(tc.tile_pool(name="ypool", bufs=6))
        p2pool = ctx.enter_context(tc.tile_pool(name="p2pool", bufs=4))
        pgemm = ctx.enter_context(tc.tile_pool(name="pgemm", bufs=2, space="PSUM"))
        pxtg = ctx.enter_context(tc.tile_pool(name="pxtg", bufs=2, space="PSUM"))
        pout = ctx.enter_context(tc.tile_pool(name="pout", bufs=1, space="PSUM"))

        ident = const.tile([128, 128], F32)
        make_identity(nc, ident[:])

        xgidx_t = const.tile([128, 60], I32)
        nc.sync.dma_start(xgidx_t[:], xgidx_d[:])


        # persistent output accumulators (one PSUM bank each)
        outT_ps = [
            pout.tile([128, BS], F32, tag=f"pout{oh}", name=f"pout{oh}")
            for oh in range(2)
        ]

        # ---- Phase 0: load x shard, transpose to xT (scaled) ----
        xs0 = const.tile([128, X], F32)  # batch rows 0..127
        xs1 = const.tile([128, X], F32)  # batch rows 128..255
        nc.sync.dma_start(xs0[:], xs_d[0:128, :])
        nc.sync.dma_start(xs1[:], xs_d[128:256, :])

        xts = const.tile([128, 512], FP16)  # [p, c*256+b] = 0.1*xT[c*128+p, b]
        for ih in range(2):
            for bh, xsrc in ((0, xs0), (1, xs1)):
                pst = pxtg.tile([128, 128], F32, tag="xtg", name="pst")
                nc.tensor.transpose(
                    pst[:], xsrc[:, ih * 128 : (ih + 1) * 128], ident[:]
                )
                nc.vector.tensor_scalar_mul(
                    xts[:, ih * 256 + bh * 128 : ih * 256 + (bh + 1) * 128],
                    pst[:],
                    0.1,
                )

        # ---- Phase 1a: gather x_rep tiles from xT via indirect DMA ----
        def igather(dst_slice, col):
            nc.gpsimd.indirect_dma_start(
                out=dst_slice,
                out_offset=None,
                in_=xt_d[:],
                in_offset=IndirectOffsetOnAxis(ap=xgidx_t[:, col : col + 1], axis=0),
            )

        xrep0 = [const.tile([128, 256], FP16, tag=f"xrep0_{kc}", name=f"xrep0_{kc}") for kc in range(4)]
        for kc in range(4):
            igather(xrep0[kc][:], kc)
        xrep1 = [const.tile([128, 1024], FP16, tag=f"xrep1_{kc}", name=f"xrep1_{kc}") for kc in range(2)]
        for kc in range(2):
            for rho in range(4):
                igather(xrep1[kc][:, rho * 256 : (rho + 1) * 256], 4 + kc * 4 + rho)
        xrep2 = const.tile([128, 4096], FP16)
        for rho in range(16):
            igather(xrep2[:, rho * 256 : (rho + 1) * 256], 12 + rho)
        xrep3 = const.tile([128, 8192], FP16)
        for q in range(32):
            igather(xrep3[:, q * 256 : (q + 1) * 256], 28 + q)

        # ---- fused phase 2 machinery ----
        state = {"c": 0, "rt_lo": None, "rt_hi": None, "oht": None, "pend": []}

        def flush_pending():
            for obld, z, c in state["pend"]:
                for oh2 in range(2):
                    nc.tensor.matmul(
                        outT_ps[oh2][:],
                        obld[:, oh2 * 128 : (oh2 + 1) * 128],
                        z[:],
                        start=(c == 0),
                        stop=(c == NCHUNK - 1),
                        skip_group_check=True,
                    )
            state["pend"] = []

        def fuse_chunk(ysrc):
            """Consume one (128 k, 256 b) Y slice: xT-gather via RT matmul,
            Z = Y*xTg, O-matmul scatter-reduce into outT_ps."""
            c = state["c"]
            state["c"] = c + 1
            if len(state["pend"]) >= 8:
                flush_pending()
            if c % 8 == 0:
                rt_lo = rtpool.tile([128, 1024], FP16, tag="rt_lo", name="rt_lo")
                rt_hi = rtpool.tile([128, 1024], FP16, tag="rt_hi", name="rt_hi")
                nc.sync.dma_start(rt_lo[:], rt_d[0:128, c * 128 : (c + 8) * 128])
                nc.sync.dma_start(rt_hi[:], rt_d[128:256, c * 128 : (c + 8) * 128])
                state["rt_lo"], state["rt_hi"] = rt_lo, rt_hi
                oht = rtpool.tile([128, 2048], FP16, tag="oht", name="oht")
                nc.sync.dma_start(oht[:], oh_d[:, c * 256 : (c + 8) * 256])
                state["oht"] = oht
            rts = (state["rt_lo"], state["rt_hi"])
            lo = (c % 8) * 128
            xtg = pxtg.tile([128, 256], F32, tag="xtg", name="xtg")
            for ih in range(2):
                nc.tensor.matmul(
                    xtg[:],
                    rts[ih][:, lo : lo + 128],
                    xts[:, ih * 256 : (ih + 1) * 256],
                    start=(ih == 0),
                    stop=(ih == 1),
                )
            obld = state["oht"][:, (c % 8) * 256 : (c % 8 + 1) * 256]
            z = p2pool.tile([128, 256], FP16, tag="z", name="z", bufs=14)
            nc.vector.tensor_mul(z[:], ysrc, xtg[:])
            state["pend"].append((obld, z, c))

        copy_eng = [lambda o, i: nc.vector.tensor_copy(o, i),
                    lambda o, i: nc.scalar.copy(o, i)]
        state_cp = {"n": 0}

        def psum_to_sbuf(ps_ap):
            yt = ypool.tile([128, 1024], F32, tag="ytile", name="yt")
            copy_eng[state_cp["n"] % 2](yt[:], ps_ap)
            state_cp["n"] += 1
            return yt

        # ---- Phase 1b+2: group GEMMs with fused consumption ----
        # g0: k = mu
        for mp in range(16):
            w0t = [wpool.tile([128, 1024], FP16, tag=f"w0_{kc}", name=f"w0_{kc}") for kc in range(4)]
            for kc in range(4):
                nc.sync.dma_start(
                    w0t[kc][:],
                    wt_d[0][kc * 128 : (kc + 1) * 128, mp * 1024 : (mp + 1) * 1024],
                )
            for mq in range(2):
                ps = pgemm.tile([128, 1024], F32, tag="pg", name="ps")
                for sub in range(4):
                    mt = mq * 4 + sub
                    for kc in range(4):
                        nc.tensor.matmul(
                            ps[:, sub * 256 : (sub + 1) * 256],
                            w0t[kc][:, mt * 128 : (mt + 1) * 128],
                            xrep0[kc][:],
                            start=(kc == 0),
                            stop=(kc == 3),
                        )
                yt = psum_to_sbuf(ps[:])
                for sub in range(4):
                    fuse_chunk(yt[:, sub * 256 : (sub + 1) * 256])

        # g1: k = 16384 + mu*4 + rho
        w1t = [const.tile([128, 4096], FP16, tag=f"w1_{kc}", name=f"w1_{kc}") for kc in range(2)]
        for kc in range(2):
            nc.sync.dma_start(w1t[kc][:], wt_d[1][kc * 128 : (kc + 1) * 128, :])
        for mt in range(32):
            ps = pgemm.tile([128, 1024], F32, tag="pg", name="ps")
            for ns in range(2):
                for kc in range(2):
                    nc.tensor.matmul(
                        ps[:, ns * 512 : (ns + 1) * 512],
                        w1t[kc][:, mt * 128 : (mt + 1) * 128],
                        xrep1[kc][:, ns * 512 : (ns + 1) * 512],
                        start=(kc == 0),
                        stop=(kc == 1),
                    )
            yt = psum_to_sbuf(ps[:])
            for j in range(4):
                fuse_chunk(yt[:, j * 256 : (j + 1) * 256])

        # g2: k = 32768 + mu*16 + rho
        w2t = const.tile([128, 1024], FP16)
        nc.sync.dma_start(w2t[:], wt_d[2][:])
        for mt in range(8):
            for np2 in range(4):
                ps = pgemm.tile([128, 1024], F32, tag="pg", name="ps")
                for sub in range(2):
                    ns = np2 * 2 + sub
                    nc.tensor.matmul(
                        ps[:, sub * 512 : (sub + 1) * 512],
                        w2t[:, mt * 128 : (mt + 1) * 128],
                        xrep2[:, ns * 512 : (ns + 1) * 512],
                        start=True,
                        stop=True,
                    )
                yt = psum_to_sbuf(ps[:])
                for j in range(4):
                    fuse_chunk(yt[:, j * 256 : (j + 1) * 256])

        # g3: k = 49152 + mu*64 + 2q + s
        w3t = const.tile([128, 256], FP16)
        nc.sync.dma_start(w3t[:], wt_d[3][:])
        for mt in range(2):
            for s in range(2):
                for np3 in range(8):
                    ps = pgemm.tile([128, 1024], F32, tag="pg", name="ps")
                    for sub in range(2):
                        ns = np3 * 2 + sub
                        nc.tensor.matmul(
                            ps[:, sub * 512 : (sub + 1) * 512],
                            w3t[s * 64 : (s + 1) * 64, mt * 128 : (mt + 1) * 128],
                            xrep3[s * 64 : (s + 1) * 64, ns * 512 : (ns + 1) * 512],
                            start=True,
                            stop=True,
                        )
                    yt = psum_to_sbuf(ps[:])
                    for j in range(4):
                        fuse_chunk(yt[:, j * 256 : (j + 1) * 256])

        assert state["c"] == NCHUNK
        flush_pending()

        # ---- epilogue: outT (o, b) -> out (b, o) ----
        outstage = [const.tile([128, 256], F32, tag=f"outstage{bh}", name=f"outstage{bh}") for bh in range(2)]
        for oh in range(2):
            outT_sb = p2pool.tile([128, 256], F32, tag="outT_sb", name="outT_sb", bufs=2)
            nc.vector.tensor_copy(outT_sb[:], outT_ps[oh][:])
            for bh in range(2):
                pst2 = pxtg.tile([128, 128], F32, tag="xtg", name="pst2")
                nc.tensor.transpose(
                    pst2[:], outT_sb[:, bh * 128 : (bh + 1) * 128], ident[:]
                )
                nc.any.tensor_copy(
                    outstage[bh][:, oh * 128 : (oh + 1) * 128], pst2[:]
                )
        for bh in range(2):
            nc.sync.dma_start(out_d[bh * 128 : (bh + 1) * 128, :], outstage[bh][:])

    nc.compile()
    return nc


_NC_CACHE = None


def _make_in_maps(x, wt0, wt1, wt2, wt3, xgidx, rt, oh):
    x = np.ascontiguousarray(np.asarray(x, dtype=np.float32))
    in_maps = []
    for c in range(NCORES):
        xsh = x[c * BS : (c + 1) * BS, :]
        in_maps.append(
            {
                "xs": xsh,
                "xt": np.ascontiguousarray(xsh.T.astype(np.float16)),
                "wt0": wt0,
                "wt1": wt1,
                "wt2": wt2,
                "wt3": wt3,
                "xgidx": xgidx,
                "rt": rt,
                "oh": oh,
            }
        )
    return in_maps


def kernel(x, W0, W1, W2, W3, bid0, bid1, bid2, bid3, W_invperm, **_unused):
    global _NC_CACHE
    prep = _host_prep(W0, W1, W2, W3, bid0, bid1, bid2, bid3, W_invperm)
    if _NC_CACHE is None:
        _NC_CACHE = _build_nc()
    nc = _NC_CACHE

    in_maps = _make_in_maps(x, *prep)
    res = run_bass_kernel_spmd(nc, in_maps, core_ids=list(range(NCORES)))
    out = np.concatenate([res.results[c]["out"] for c in range(NCORES)], axis=0)
    return out.astype(np.float32)
